# revision 1
# baseline (speedup 1.0000x reference)
"""Trainium2 Bass kernel v2 for nn_BSQLinear (vq_codebook).

Reference computes:
    stacked = einsum('npl,plc->npc', vq_weight, w_dec) + b_dec     # (16384,4,256)
    w_flat  = stacked.transpose(1,0,2).reshape(4,-1)*(d_std+eps)+d_mean
    w_recon = w_flat.reshape(4,1024,4096).reshape(4096,4096)
    out     = x @ w_recon.T + bias                                  # (4,2048,4096)

Index algebra: with o = p*1024 + o_sub, i = ns*256 + c, n = o_sub*16 + ns:
    w_recon[o, i] = sum_l vq[n,p,l]*wdec'[p,l,c] + b'[p,c]
        wdec' = (d_std+eps)*w_dec,  b' = (d_std+eps)*b_dec + d_mean
so the 274-GFLOP GEMM factorizes through the rank-32 bottleneck:
    Y[t,p,ns,l] = sum_c x[t, ns*256+c] * wdec'[p,l,c]          (stage 1)
    out[t,o]    = sum_{ns,l} Y[t,p,ns,l]*vq[o_sub*16+ns,p,l]   (stage 2)
                  + S[t,p] + bias[o]
    S[t,p]      = sum_{ns,c} x[t,ns*256+c] * b'[p,c]           (S pass)

v2 changes vs v1:
  - x is transposed on the HOST -> xt tiles [c-par, t-free] DMA straight
    in; no PE transposes, no transpose-evac copies.
  - fp16 on device for x / vq2 / Y / out (error budget 2e-2, fp16 adds
    ~1e-3); halves both input and output HBM traffic.
  - stage 1 uses the unpadded stationary [128c, 128(p,l)] shared by all
    16 ns-chunks (4x fewer stage-1 matmul cycles than the zero-padded
    v1 form). The [(p,l),t] -> per-p [(ns%4)*32+l, t] permute rides the
    PSUM evacuation as 4x [32,512] copies spread over vector/scalar/
    gpsimd engines.

Sharding: data-parallel over the 8192 tokens -> 1024 tokens/core.
"""

import os
from contextlib import ExitStack

import numpy as np

import concourse.bacc as bacc
import concourse.bass as bass
import concourse.mybir as mybir
import concourse.tile as tile
from concourse.bass_utils import run_bass_kernel_spmd

P = 4
OUT_PER = 1024
IN_F = 4096
OUT_F = 4096
EPS = 1e-6
N_CORES = 8
T_TOTAL = 8192
TC = T_TOTAL // N_CORES  # 1024 tokens per core

F32 = mybir.dt.float32
F16 = mybir.dt.float16
NP16 = np.float16

LAST_RESULTS = None  # BassKernelResults from the most recent run (for test.py)


def _build_bass(loop_n: int | None = None):
    nc = bacc.Bacc(None, target_bir_lowering=False)

    # xt: host-transposed x slice, tiled [32 c-chunks][128 c][1024 t]
    xt_d = nc.dram_tensor("xt", [32, 128, TC], F16, kind="ExternalInput")
    # w2[ch][cc][p*32+l] = wdec'[p, l, ch*128+cc]
    w2_d = nc.dram_tensor("w2", [2, 128, 128], F16, kind="ExternalInput")
    # vq2[p][kc][(ns%4)*32+l][o_sub] = vq[o_sub*16+ns, p, l], ns = kc*4+(ns%4)
    vq2_d = nc.dram_tensor("vq2", [4, 4, 128, 1024], F16, kind="ExternalInput")
    # bpt[ch][cc][p] = b'[p, ch*128+cc]
    bpt_d = nc.dram_tensor("bpt", [2, 128, 4], F16, kind="ExternalInput")
    ident4_d = nc.dram_tensor("ident4", [4, 4], F32, kind="ExternalInput")
    biasrep_d = nc.dram_tensor("biasrep", [128, OUT_F], F32, kind="ExternalInput")
    out_d = nc.dram_tensor("out", [TC, OUT_F], F16, kind="ExternalOutput")

    with tile.TileContext(nc) as tc, ExitStack() as ctx:
        cpool = ctx.enter_context(tc.tile_pool(name="consts", bufs=1))
        ypool = ctx.enter_context(tc.tile_pool(name="y", bufs=1))
        xpool = ctx.enter_context(tc.tile_pool(name="x", bufs=16))
        opool = ctx.enter_context(tc.tile_pool(name="osb", bufs=4))
        spool = ctx.enter_context(tc.tile_pool(name="s", bufs=2))
        pp_y = ctx.enter_context(tc.tile_pool(name="ppy", bufs=4, space="PSUM"))
        pp_o = ctx.enter_context(tc.tile_pool(name="ppo", bufs=2, space="PSUM"))
        pp_s = ctx.enter_context(tc.tile_pool(name="pps", bufs=1, space="PSUM"))
        pp_ss = ctx.enter_context(tc.tile_pool(name="ppss", bufs=1, space="PSUM"))

        # ---- resident constants ----
        w2_sb = []
        for ch in range(2):
            t = cpool.tile([128, 128], F16, tag=f"w2{ch}", name=f"w2{ch}")
            nc.sync.dma_start(out=t[:, :], in_=w2_d[ch])
            w2_sb.append(t)
        bpt_sb = []
        for ch in range(2):
            t = cpool.tile([128, 4], F16, tag=f"bpt{ch}", name=f"bpt{ch}")
            nc.sync.dma_start(out=t[:, :], in_=bpt_d[ch])
            bpt_sb.append(t)
        ident4_sb = cpool.tile([4, 4], F32, tag="ident4")
        nc.sync.dma_start(out=ident4_sb[:, :], in_=ident4_d[:, :])
        biasrep_sb = cpool.tile([128, OUT_F], F32, tag="biasrep")
        nc.sync.dma_start(out=biasrep_sb[:, :], in_=biasrep_d[:, :])
        vq2_sb = {}
        for p in range(4):
            for kc in range(4):
                t = cpool.tile([128, 1024], F16, tag=f"vq{p}{kc}", name=f"vq{p}{kc}")
                nc.sync.dma_start(out=t[:, :], in_=vq2_d[p, kc])
                vq2_sb[(p, kc)] = t

        # ---- persistent Y tiles: per (p, kc) and half: [128=(4ns x 32l), 512t]
        y_sb = {}
        for p in range(4):
            for kc in range(4):
                for hh in range(2):
                    y_sb[(p, kc, hh)] = ypool.tile(
                        [128, 512], F16, tag=f"y{p}{kc}{hh}", name=f"y{p}{kc}{hh}"
                    )
        s2_sb = [
            spool.tile([128, 4], F32, tag=f"s2_{i}", bufs=1, name=f"s2_{i}")
            for i in range(8)
        ]

        # engines to spread the permute-evac copies over. Only DVE and ACT
        # can read PSUM (gpsimd cannot); DVE also carries the 64 stage-2
        # stt evacuations, so give ACT 2/3 of the permute copies.
        copy_fns = [nc.vector.tensor_copy, nc.scalar.copy, nc.scalar.copy]

        loop_ctx = tc.For_i(0, loop_n, 1) if loop_n else None
        if loop_ctx is not None:
            ctx.enter_context(loop_ctx)

        for h in range(2):
            # S^T accumulator for this half: [p=4, t=512]
            ps_st = pp_s.tile([4, 512], F32, tag="st")
            for kc in range(4):  # i-quarter; covers ns in [4*kc, 4*kc+4)
                # load the 8 x tiles of this i-quarter (both halves on h==0
                # would double SBUF; keep per-half slices of the DMA)
                xtiles = {}
                for nsq in range(4):
                    for ch in range(2):
                        g = kc * 8 + nsq * 2 + ch
                        t = xpool.tile(
                            [128, 512], F16, tag="xt", name=f"x_{h}_{g}"
                        )
                        nc.sync.dma_start(
                            out=t[:, :], in_=xt_d[g, :, h * 512 : (h + 1) * 512]
                        )
                        xtiles[(nsq, ch)] = t

                for nsq in range(4):
                    # stage 1 (unpadded): py[(p,l), t] for ns = kc*4+nsq
                    py = pp_y.tile([128, 512], F32, tag="py")
                    for ch in range(2):
                        nc.tensor.matmul(
                            py[:, :],
                            w2_sb[ch][:, :],
                            xtiles[(nsq, ch)][:, :],
                            start=(ch == 0),
                            stop=(ch == 1),
                        )
                    # permute-evac: rows p*32..p*32+32 -> y_sb[(p,kc,h)]
                    # rows nsq*32..nsq*32+32, spread over 3 engines
                    for p in range(4):
                        fn = copy_fns[(nsq * 4 + p) % 3]
                        fn(
                            y_sb[(p, kc, h)][nsq * 32 : (nsq + 1) * 32, :],
                            py[p * 32 : (p + 1) * 32, :],
                        )

                # S pass: accumulate b'^T x over every i-chunk of this half
                for nsq in range(4):
                    for ch in range(2):
                        nc.tensor.matmul(
                            ps_st[:, :],
                            bpt_sb[ch][:, :],
                            xtiles[(nsq, ch)][:, :],
                            start=(kc == 0 and nsq == 0 and ch == 0),
                            stop=(kc == 3 and nsq == 3 and ch == 1),
                            skip_group_check=True,
                        )

            # finalize S for this half: evict, transpose [4,128]->[128,4]
            st_sb = spool.tile([4, 512], F32, tag="stsb", bufs=1)
            nc.vector.tensor_copy(st_sb[:, :], ps_st[:, :])
            for tc4 in range(4):
                pss = pp_ss.tile([128, 4], F32, tag="pss")
                nc.tensor.matmul(
                    pss[:, :],
                    st_sb[:, tc4 * 128 : (tc4 + 1) * 128],
                    ident4_sb[:, :],
                    is_transpose=True,
                    start=True,
                    stop=True,
                    skip_group_check=True,
                )
                nc.scalar.copy(s2_sb[h * 4 + tc4][:, :], pss[:, :])

            # stage 2 for this half
            for p in range(4):
                for tm in range(4):
                    osb = opool.tile(
                        [128, 1024], F16, tag="osb", name=f"osb_{h}_{p}_{tm}"
                    )
                    for oh in range(2):
                        po = pp_o.tile([128, 512], F32, tag="po")
                        for kc in range(4):
                            nc.tensor.matmul(
                                po[:, :],
                                y_sb[(p, kc, h)][:, tm * 128 : (tm + 1) * 128],
                                vq2_sb[(p, kc)][:, oh * 512 : (oh + 1) * 512],
                                start=(kc == 0),
                                stop=(kc == 3),
                            )
                        # out = (psum + S[t,p]) + bias[o]
                        nc.vector.scalar_tensor_tensor(
                            osb[:, oh * 512 : (oh + 1) * 512],
                            po[:, :],
                            s2_sb[h * 4 + tm][:, p : p + 1],
                            biasrep_sb[:, p * 1024 + oh * 512 : p * 1024 + (oh + 1) * 512],
                            op0=mybir.AluOpType.add,
                            op1=mybir.AluOpType.add,
                        )
                    nc.scalar.dma_start(
                        out=out_d[
                            h * 512 + tm * 128 : h * 512 + (tm + 1) * 128,
                            p * 1024 : (p + 1) * 1024,
                        ],
                        in_=osb[:, :],
                    )

    nc.compile()
    return nc


_NC_CACHE = {}


def _get_nc(loop_n=None):
    if loop_n not in _NC_CACHE:
        _NC_CACHE[loop_n] = _build_bass(loop_n)
    return _NC_CACHE[loop_n]


def _host_prep(x, vq_weight, w_dec, b_dec, d_mean, d_std, bias):
    f4 = np.float32
    x2 = np.asarray(x, dtype=f4).reshape(T_TOTAL, IN_F)
    scale = (np.asarray(d_std, f4) + EPS)  # (4,1)
    wdecp = (np.asarray(w_dec, f4) * scale[:, :, None])  # (4,32,256)
    bp = (np.asarray(b_dec, f4) * scale + np.asarray(d_mean, f4))  # (4,256)

    # w2[ch][cc][p*32+l] = wdec'[p,l,ch*128+cc]
    w2 = np.ascontiguousarray(
        wdecp.reshape(4 * 32, 256).T.reshape(2, 128, 128), dtype=NP16
    )

    # vq2[p][kc][(ns%4)*32+l][o_sub] = vq[o_sub*16+ns, p, l]
    vq2 = np.ascontiguousarray(
        np.asarray(vq_weight, f4)
        .reshape(1024, 16, 4, 32)
        .transpose(2, 1, 3, 0)
        .reshape(4, 4, 128, 1024),
        dtype=NP16,
    )
    bpt = np.ascontiguousarray(bp.T.reshape(2, 128, 4), dtype=NP16)  # [ch][cc][p]
    identity4 = np.eye(4, dtype=f4)
    biasrep = np.ascontiguousarray(
        np.broadcast_to(np.asarray(bias, f4), (128, OUT_F))
    )
    # per-core transposed x tiles: [32, 128, TC] fp16
    x16 = x2.astype(NP16)
    return x16, w2, vq2, bpt, identity4, biasrep


def _make_in_maps(x16, w2, vq2, bpt, identity4, biasrep):
    in_maps = []
    for k in range(N_CORES):
        xtk = np.ascontiguousarray(x16[k * TC : (k + 1) * TC].T).reshape(32, 128, TC)
        in_maps.append(
            {
                "xt": xtk,
                "w2": w2,
                "vq2": vq2,
                "bpt": bpt,
                "ident4": identity4,
                "biasrep": biasrep,
            }
        )
    return in_maps


def kernel(x, vq_weight, w_dec, b_dec, d_mean, d_std, bias, loop_n=None):
    global LAST_RESULTS
    prep = _host_prep(x, vq_weight, w_dec, b_dec, d_mean, d_std, bias)
    nc = _get_nc(loop_n if loop_n is not None else 1)
    in_maps = _make_in_maps(*prep)
    res = run_bass_kernel_spmd(nc, in_maps, list(range(N_CORES)), trace=False)
    LAST_RESULTS = res
    out = np.concatenate([res.results[k]["out"] for k in range(N_CORES)], axis=0)
    return out.reshape(4, 2048, OUT_F).astype(np.float32)

